# revision 1
# baseline (speedup 1.0000x reference)
"""DiffusionGraphConv Trainium2 kernel (8-core SPMD, data-parallel over batch).

Math refactoring (halves the big-matmul FLOPs vs the reference order):
  reference: out[b,n,o] = sum_{f,m} mats_m[n,f,b] * W[f*5+m, o]
  with mats = [x0, s0 x0, 2 s0^2 x0 - x0, s1 x0, 2 s1^2 x0 - x0].
  Projection (width F=128 -> O=64) commutes with the node-space diffusion, so:
    u_m = proj(x0, W_m)                       # [N, O, B] each, cheap
    out = (u0 - u2 - u4) + s0 (u1 + 2 s0 u2) + s1 (u3 + 2 s1 u4)
  Device computes: v0 = proj(x0, W0-W2-W4), and with pre-scaled 2*W2 / 2*W4:
    c0 = u1 + s0 @ u2s ; c1 = u3 + s1 @ u4s ; out = v0 + s0 @ c0 + s1 @ c1

Per-core work: 4 matmuls [2048,2048]@[2048,512] (bf16, fp32 PSUM) + projections.
Layouts (host-prepared, all "SBUF images"):
  x0t  [128 f, 16t*8b*128j] bf16: x0t[f, (t*8+b)*128+j] = cat(inputs,state)[b, t*128+j, f]
  wcat [128 f, 5*64]        bf16: [W0-W2-W4 | W1 | 2*W2 | W3 | 2*W4]
  s*t  [16 t, 128 p, 2048]  bf16: s*t[t, p, kt*128+j] = s[t*128+j, kt*128+p]
       (strip t = transposed rows of s for output-node tile t, k-major)
  out  [2048 n, 8b*64o]     f32

Env quirks handled here: walrus accepts <=1 sync-wait per instruction
(_legalize_waits hoists extras onto EventSemaphore carriers; simulators need
legalize=False); repeat=N re-runs the idempotent pipeline for wall-clock
differencing since this axon terminal has no NTFF profiling.
"""

import sys

if "/opt/trn_rl_repo" not in sys.path:
    sys.path.insert(0, "/opt/trn_rl_repo")

import numpy as np
import ml_dtypes

import concourse.bass as bass
import concourse.mybir as mybir
from concourse.tile import TileContext
from concourse.bass_utils import run_bass_kernel_spmd

BF16 = mybir.dt.bfloat16
FP8 = mybir.dt.float8e4
NPFP8 = ml_dtypes.float8_e4m3
SCALE = 256.0
F32 = mybir.dt.float32
NPBF16 = ml_dtypes.bfloat16

N = 2048          # graph nodes
F = 128           # input_size (64 input + 64 hidden)
B = 64            # global batch
NCORES = 8
BS = B // NCORES  # 8 batches per core
O = 64            # output features
NT = N // 128     # 16 node tiles
M5 = 5            # diffusion matrices


def _legalize_waits(nc, max_waits=1):
    """Walrus in this env encodes at most one sync-wait per instruction.

    Tile's sem assignment can emit 2-3 waits on one instruction; hoist the
    excess onto standalone EventSemaphore carriers (same engine, inserted
    just before), which the sequencer executes in order — semantics are
    identical, encoding is legal."""
    f = nc.m.functions[0]
    for blk in f.blocks:
        new_insts = []
        changed = False
        for inst in blk.instructions:
            si = inst.sync_info
            waits = list(si.on_wait) if si is not None else []
            if len(waits) > max_waits:
                for i, w in enumerate(waits[:-max_waits]):
                    ev = mybir.InstEventSemaphore(
                        name=f"{inst.name}-wsplit{i}",
                        engine=inst.engine,
                        ins=[],
                        outs=[],
                        sync_info=mybir.SyncInfo(on_wait=[w], on_update=[]),
                    )
                    new_insts.append(ev)
                inst.sync_info = mybir.SyncInfo(
                    on_wait=waits[-max_waits:], on_update=list(si.on_update)
                )
                changed = True
            new_insts.append(inst)
        if changed:
            blk.instructions = new_insts
    return nc


def build_bass(n=N, bs=BS, o=O, legalize=True, n_hops=4, repeat=1):
    """Build the per-core SPMD Bass program."""
    nt = n // 128
    nc = bass.Bass()
    x0t = nc.dram_tensor("x0t", [F, bs * n], BF16, kind="ExternalInput")
    wcat = nc.dram_tensor("wcat", [F, M5 * o], BF16, kind="ExternalInput")
    s0t = nc.dram_tensor("s0t", [nt, 128, n], FP8, kind="ExternalInput")
    s1t = nc.dram_tensor("s1t", [nt, 128, n], FP8, kind="ExternalInput")
    out = nc.dram_tensor("out", [n, bs * o], F32, kind="ExternalOutput")

    obs = bs * o        # 512: width of diffusion operands
    with TileContext(nc) as tc:
        with (
            tc.tile_pool(name="persist", bufs=1) as persist,
            tc.tile_pool(name="stream", bufs=6) as stream,
            tc.tile_pool(name="pproj", bufs=4, space="PSUM") as pproj,
            tc.tile_pool(name="pacc", bufs=4, space="PSUM") as pacc,
        ):
            w_sb = persist.tile([F, M5 * o], BF16, name="w_sb")
            nc.sync.dma_start(out=w_sb[:, :], in_=wcat[:, :])
            # x0t is t-major on host: free index = t*bs*128 + b*128 + j, so
            # each node-tile's stationary slices arrive in one chunk DMA.
            x0_sb = persist.tile([F, bs * n], BF16, name="x0_sb")
            for t in range(nt):
                nc.sync.dma_start(
                    out=x0_sb[:, t * bs * 128:(t + 1) * bs * 128],
                    in_=x0t[:, t * bs * 128:(t + 1) * bs * 128],
                )
            # U[t]: [128, bs*5*o] bf16, b-major: free = b*320 + mi*64 + oo.
            # Slots mi: 0=v0, 1=u1->c0, 2=2*u2, 3=u3->c1, 4=2*u4.
            U = [
                persist.tile([128, 4 * 2 * obs], FP8, name=f"u{tp}", tag=f"u{tp}")
                for tp in range(nt // 2)
            ]
            V0 = [
                persist.tile([128, obs], BF16, name=f"w0_{t}", tag=f"w0_{t}")
                for t in range(nt)
            ]
            # V[t]: [128, obs] f32 accumulator, created in the first V-hop.
            V = [
                persist.tile([128, obs], F32, name=f"v{t}", tag=f"v{t}")
                for t in range(nt)
            ]

            def upair(tp, mi):
                """[128, 2, obs] DoubleRow moving view: k-tile pair of slot mi."""
                return U[tp].rearrange("p (mi4 kt2 c) -> p mi4 kt2 c", mi4=4, kt2=2)[
                    :, mi - 1, :, :
                ]

            def uslot_w(t, mi):
                """[128, obs] contiguous write view of slot mi for node-tile t."""
                base = (mi - 1) * 2 * obs + (t % 2) * obs
                return U[t // 2][:, base:base + obs]

            # ---- Phase 1: projections, node-tile outer so U[t] completes
            # early and hop-1 PSUM groups can close while P1 still runs.
            #   psum[:, h*512 : h*512+320] = x0_tile(b).T @ wcat   (n on psum partitions)
            def phase1(t):
                # one-bank psum per b (bufs=4 rotation) keeps PE from stalling
                # on copy drains; copies alternate DVE/ACT to pipeline at 2x
                for b in range(bs):
                    ps = pproj.tile([128, 512], F32, name="ps_proj", tag="proj")
                    nc.tensor.matmul(
                        ps[:, 0:M5 * o],
                        lhsT=x0_sb[:, (t * bs + b) * 128:(t * bs + b + 1) * 128],
                        rhs=w_sb[:, :],
                        start=True,
                        stop=True,
                    )
                    du = U[t // 2].rearrange(
                        "p (mi4 kt2 c) -> p mi4 kt2 c", mi4=4, kt2=2
                    )[:, :, t % 2, b * o:(b + 1) * o]
                    su = ps[:, o:M5 * o].rearrange("p (mi4 oo) -> p mi4 oo", oo=o)
                    if b % 2 == 0:
                        nc.vector.tensor_copy(out=V0[t][:, b * o:(b + 1) * o], in_=ps[:, 0:o])
                        nc.vector.tensor_copy(out=du, in_=su)
                    else:
                        nc.scalar.copy(out=V0[t][:, b * o:(b + 1) * o], in_=ps[:, 0:o])
                        nc.scalar.copy(out=du, in_=su)

            # ---- Phases 2-5: diffusion hops.
            #   hop(s, src_slot, dst):  for each node-tile t:
            #     acc = sum_kt sT_strip[t,kt].T @ U[kt][src_slot]   (= (s @ u)[t-tile])
            def hop(sdram, src, dst_slot, first_v, final, split_k=1):
                for t in range(nt):
                    strip = stream.tile([128, n], FP8, name="strip", tag="strip")
                    nc.sync.dma_start(out=strip[:, :], in_=sdram[t])
                    # split_k>1: independent psum sub-groups over kt ranges, so
                    # early sub-groups can close while upstream U tiles are
                    # still being produced (fills PE idle at phase boundaries)
                    pss = []
                    ps = pacc.tile([128, obs], F32, name="ps_acc", tag="acc")
                    for ktp in range(nt // 2):
                        nc.tensor.matmul(
                            ps[:, :],
                            lhsT=strip[:, ktp * 256:(ktp + 1) * 256].rearrange(
                                "p (kt2 j) -> p kt2 j", kt2=2),
                            rhs=upair(ktp, src),
                            start=(ktp == 0),
                            stop=(ktp == nt // 2 - 1),
                            perf_mode=mybir.MatmulPerfMode.DoubleRow,
                        )
                    pss.append(ps)
                    if first_v:
                        # V = v0 + s0 @ c0   (V layout: b*o + oo, matches psum)
                        nc.vector.tensor_add(V[t][:, :], pss[0][:, :], uslot(t, 0))
                        for ps in pss[1:]:
                            nc.vector.tensor_add(V[t][:, :], V[t][:, :], ps[:, :])
                    elif final:
                        for ps in pss:
                            nc.vector.tensor_add(V[t][:, :], V[t][:, :], ps[:, :])
                        nc.sync.dma_start(
                            out=out[t * 128:(t + 1) * 128, :], in_=V[t][:, :]
                        )
                    else:
                        # psum = (256*s0)@(2u2/16) = 16*(2 s0 u2); slot1 = 16*u1
                        # -> plain add keeps c0 at 16x scale (fp8-safe)
                        d = uslot_w(t, dst_slot)
                        for ps in pss:
                            nc.vector.tensor_add(d, d, ps[:, :])

            hops = [
                (s0t, 2, 1, False, False, 1),    # c0 = u1 + s0 @ (2 u2)
                (s1t, 4, 3, False, False, 1),    # c1 = u3 + s1 @ (2 u4)
                (s0t, 1, None, True, False, 1),  # V = v0 + s0 @ c0
                (s1t, 3, None, False, True, 1),  # V += s1 @ c1 ; dma out
            ]
            # Final phase: hops 3+4 merged into one 32-matmul accumulation
            # per output tile (V = v0 + s0@c0 + s1@c1 with a single psum
            # group) — fewer adds, V written once, then streamed out.
            def final_merged():
                for t in range(nt):
                    strip0 = stream.tile([128, n], FP8, name="strip", tag="strip")
                    nc.sync.dma_start(out=strip0[:, :], in_=s0t[t])
                    strip1 = stream.tile([128, n], FP8, name="strip", tag="strip")
                    nc.sync.dma_start(out=strip1[:, :], in_=s1t[t])
                    ps = pacc.tile([128, obs], F32, name="ps_acc", tag="acc")
                    for g, (sb, sl) in enumerate([(strip0, 1), (strip1, 3)]):
                        for ktp in range(nt // 2):
                            nc.tensor.matmul(
                                ps[:, :],
                                lhsT=sb[:, ktp * 256:(ktp + 1) * 256].rearrange(
                                    "p (kt2 j) -> p kt2 j", kt2=2),
                                rhs=upair(ktp, sl),
                                start=(g == 0 and ktp == 0),
                                stop=(g == 1 and ktp == nt // 2 - 1),
                                perf_mode=mybir.MatmulPerfMode.DoubleRow,
                            )
                    nc.vector.scalar_tensor_tensor(
                        out=V[t][:, :], in0=ps[:, :], scalar=1.0 / (SCALE * 16.0),
                        op0=mybir.AluOpType.mult,
                        in1=V0[t][:, :], op1=mybir.AluOpType.add)
                    nc.sync.dma_start(
                        out=out[t * 128:(t + 1) * 128, :], in_=V[t][:, :]
                    )

            # repeat>1 re-runs the whole idempotent pipeline (each round
            # rebuilds U from x0 and recreates V) — used only to measure
            # per-round device time via wall-clock differencing.
            for _rep in range(repeat):
                for t in range(nt):
                    phase1(t)
                if n_hops >= 4:
                    for hargs in hops[:2]:
                        hop(*hargs)
                    final_merged()
                else:
                    for hargs in hops[:n_hops]:
                        hop(*hargs)
    return _legalize_waits(nc) if legalize else nc


_NC_CACHE = {}


def _get_nc():
    if "nc" not in _NC_CACHE:
        _NC_CACHE["nc"] = build_bass()
    return _NC_CACHE["nc"]


def make_inputs(support0, support1, inputs, state, weight):
    """Host-side layout prep -> per-core in_maps (shared replicated arrays)."""
    xs = np.concatenate(
        [
            np.asarray(inputs, np.float32).reshape(B, N, F // 2),
            np.asarray(state, np.float32).reshape(B, N, F // 2),
        ],
        axis=2,
    )  # [B, N, F]

    w = np.asarray(weight, np.float32).reshape(F, M5, O)
    wv0 = w[:, 0] - w[:, 2] - w[:, 4]
    wcat = np.concatenate(
        [wv0, 16.0 * w[:, 1], 2.0 * w[:, 2] / 16.0,
         16.0 * w[:, 3], 2.0 * w[:, 4] / 16.0], axis=1
    ).astype(NPBF16)  # [128, 320]; hop slots scaled so fp8 adds stay in-range

    def strip_img(s):
        # fp8 DoubleRow pair layout: [t, p, ktp*256 + kt2*128 + j]
        #   = fp8(SCALE * s[t*128+j, (ktp*2+kt2)*128 + p])
        r = (SCALE * np.asarray(s, np.float32)).astype(NPFP8)
        r = r.reshape(NT, 128, NT, 128).transpose(0, 3, 2, 1)  # [t, p, kt, j]
        return np.ascontiguousarray(r.reshape(NT, 128, N))

    s0i, s1i = strip_img(support0), strip_img(support1)

    in_maps = []
    for c in range(NCORES):
        shard = xs[c * BS:(c + 1) * BS]                # [8b, N, F]
        # t-major SBUF image: x0t[f, t*BS*128 + b*128 + j] = shard[b, t*128+j, f]
        x0t = np.ascontiguousarray(
            shard.reshape(BS, NT, 128, F).transpose(3, 1, 0, 2).reshape(F, BS * N)
        ).astype(NPBF16)
        in_maps.append({"x0t": x0t, "wcat": wcat, "s0t": s0i, "s1t": s1i})
    return in_maps


def postprocess(results, biases):
    full = np.empty((B, N, O), np.float32)
    for c, r in enumerate(results):
        full[c * BS:(c + 1) * BS] = (
            r["out"].reshape(N, BS, O).transpose(1, 0, 2)
        )
    full += np.asarray(biases, np.float32)[None, None, :]
    return full.reshape(B, N * O)


def kernel(support0, support1, inputs, state, weight, biases, output_size=None,
           **run_kwargs):
    nc = _get_nc()
    in_maps = make_inputs(support0, support1, inputs, state, weight)
    res = run_bass_kernel_spmd(nc, in_maps, core_ids=list(range(NCORES)),
                               **run_kwargs)
    out = postprocess(res.results, biases)
    if run_kwargs.get("trace"):
        return out, res
    return out



# revision 7
# speedup vs baseline: 30.1111x; 30.1111x over previous
"""DiffusionGraphConv Trainium2 kernel (8-core SPMD, data-parallel over batch).

Math (halves big-matmul FLOPs vs the reference order):
  reference: out[b,n,o] = sum_{f,m} mats_m[n,f,b] * W[f*5+m, o]
  with mats = [x0, s0 x0, 2 s0^2 x0 - x0, s1 x0, 2 s1^2 x0 - x0].
  Projection (F=128 -> O=64) commutes with node-space diffusion:
    u_m = proj(x0, W_m)                       # [N, O, B] each, cheap
    v0 = proj(x0, W0-W2-W4)
    c0 = 16*u1 + (256 s0) @ (2u2/16);  c1 analogous (fp8-safe scaling)
    out = v0 + (s0 @ c0 + s1 @ c1) / (256*16)

Schedule (cost-model-driven; TimelineSim is the tuning target):
  - All input DMAs issued up front on the SP queue in arrival-deadline order
    (wcat, x0 x8 chunks, s0 halves, s1 halves). DMA transfers serialize on
    the one DMA_ENGINES resource, so order == arrival time.
  - Both supports live fully resident in SBUF (fp8 strips, 32KB/partition
    each) — loaded once, read by hops 1,2 and the final merged hop.
  - ph1a: projections for slots {2u2, 2u4}; 4 batches packed per PSUM bank
    so each bank drains with ONE strided copy (drain overhead dominates the
    DVE/ACT budget otherwise).
  - ph1b: projections for {16u1, 16u3, v0} in 2-bank PSUM megatiles (4 b
    each); emitted after ph1a and interleaved into hop1 — fills the PE idle
    while the s0 strip DMA lands.
  - hop1/hop2: per node-tile, 8 DoubleRow fp8 matmuls accumulate
    (256 s)@(2u/16); drain = in-place fp8 add onto the u1/u3 slot -> c0/c1.
  - final: per node-tile one 16-matmul group (s0@c0 + s1@c1), drained with
    scalar_tensor_tensor into bf16 V, DMA'd out per tile (bf16 out, host
    upcasts and adds biases).
  - All psum drains alternate DVE/ACT (Pool has no PSUM port).

Env quirks handled here: walrus accepts <=1 sync-wait per instruction
(_legalize_waits hoists extras onto EventSemaphore carriers; simulators need
legalize=False); repeat=N re-runs the idempotent pipeline for wall-clock
differencing since this axon terminal has no NTFF profiling.
"""

import sys

if "/opt/trn_rl_repo" not in sys.path:
    sys.path.insert(0, "/opt/trn_rl_repo")

import numpy as np
import ml_dtypes

import concourse.bass as bass
import concourse.mybir as mybir
from concourse.tile import TileContext
from concourse.bass_utils import run_bass_kernel_spmd

BF16 = mybir.dt.bfloat16
FP8 = mybir.dt.float8e4
NPFP8 = ml_dtypes.float8_e4m3
SCALE = 256.0
USCALE = 16.0
F32 = mybir.dt.float32
NPBF16 = ml_dtypes.bfloat16

N = 2048          # graph nodes
F = 128           # input_size (64 input + 64 hidden)
B = 64            # global batch
NCORES = 8
BS = B // NCORES  # 8 batches per core
O = 64            # output features
NT = N // 128     # 16 node tiles
M5 = 5            # diffusion matrices


def _legalize_waits(nc, max_waits=1):
    """Walrus in this env encodes at most one sync-wait per instruction.

    Tile's sem assignment can emit 2-3 waits on one instruction; hoist the
    excess onto standalone EventSemaphore carriers (same engine, inserted
    just before), which the sequencer executes in order — semantics are
    identical, encoding is legal."""
    f = nc.m.functions[0]
    for blk in f.blocks:
        new_insts = []
        changed = False
        for inst in blk.instructions:
            si = inst.sync_info
            waits = list(si.on_wait) if si is not None else []
            if len(waits) > max_waits:
                for i, w in enumerate(waits[:-max_waits]):
                    ev = mybir.InstEventSemaphore(
                        name=f"{inst.name}-wsplit{i}",
                        engine=inst.engine,
                        ins=[],
                        outs=[],
                        sync_info=mybir.SyncInfo(on_wait=[w], on_update=[]),
                    )
                    new_insts.append(ev)
                inst.sync_info = mybir.SyncInfo(
                    on_wait=waits[-max_waits:], on_update=list(si.on_update)
                )
                changed = True
            new_insts.append(inst)
        if changed:
            blk.instructions = new_insts
    return nc


def build_bass(n=N, bs=BS, o=O, legalize=True, repeat=1):
    """Build the per-core SPMD Bass program."""
    nt = n // 128
    obs = bs * o        # 512: width of diffusion operands
    nc = bass.Bass()
    # xw: [wcat | x0] fused so one DMA unblocks the first projection.
    # wcat: [2W2/16 | 2W4/16 | 16W1 | 16W3 | W0-W2-W4]  (320 cols)
    nw = M5 * o
    xw = nc.dram_tensor("xw", [F, nw + bs * n], BF16, kind="ExternalInput")
    s0t = nc.dram_tensor("s0t", [nt, 128, n], FP8, kind="ExternalInput")
    s1t = nc.dram_tensor("s1t", [nt, 128, n], FP8, kind="ExternalInput")
    out = nc.dram_tensor("out", [n, obs], BF16, kind="ExternalOutput")

    with TileContext(nc) as tc:
        with tc.tile_pool(name="persist", bufs=1) as persist:
            # ---- all input DMAs up front, in deadline order ----
            xw_sb = persist.tile([F, nw + bs * n], BF16, name="xw_sb")
            w_sb = xw_sb[:, 0:nw]
            nx0 = 8
            cw = bs * n // nx0
            for c in range(nx0):
                lo = 0 if c == 0 else nw + c * cw
                nc.sync.dma_start(
                    out=xw_sb[:, lo:nw + (c + 1) * cw],
                    in_=xw[:, lo:nw + (c + 1) * cw],
                )
            s0_sb = persist.tile([128, nt * n], FP8, name="s0_sb")
            s1_sb = persist.tile([128, nt * n], FP8, name="s1_sb")
            for s_sb, sdram in ((s0_sb, s0t), (s1_sb, s1t)):
                for h in range(2):
                    half = nt // 2
                    nc.sync.dma_start(
                        out=s_sb[:, h * half * n:(h + 1) * half * n].rearrange(
                            "p (t j) -> p t j", t=half),
                        in_=sdram.rearrange("t p j -> p t j")[
                            :, h * half:(h + 1) * half, :],
                    )

            # U[tp]: [128, 4 slots * 2 kt * obs] fp8.
            # slots: 0 = 2u2/16, 1 = 2u4/16, 2 = 16u1 -> c0, 3 = 16u3 -> c1.
            U = [
                persist.tile([128, 4 * 2 * obs], FP8, name=f"u{tp}", tag=f"u{tp}")
                for tp in range(nt // 2)
            ]
            V0 = [
                persist.tile([128, obs], BF16, name=f"w0_{t}", tag=f"w0_{t}")
                for t in range(nt)
            ]
            V = [
                persist.tile([128, obs], BF16, name=f"v{t}", tag=f"v{t}")
                for t in range(nt)
            ]

            def upair(tp, slot):
                """[128, 2, obs] DoubleRow moving view: k-tile pair of a slot."""
                return U[tp].rearrange(
                    "p (mi4 kt2 c) -> p mi4 kt2 c", mi4=4, kt2=2)[:, slot, :, :]

            def uslot(t, slot):
                """[128, obs] contiguous view of a slot for node-tile t."""
                base = slot * 2 * obs + (t % 2) * obs
                return U[t // 2][:, base:base + obs]

            def ub_view(t, slots, b0, nb):
                """[128, nb, len(slots), o] strided write view of U."""
                v = U[t // 2].rearrange(
                    "p (mi4 kt2 b8 o) -> p b8 mi4 kt2 o", mi4=4, kt2=2, b8=bs
                )[:, b0:b0 + nb, slots[0]:slots[-1] + 1, t % 2, :]
                return v

            def strip(s_sb, t, ktp):
                """[128, 2, 128] DoubleRow stationary view of support strip."""
                base = t * n + ktp * 256
                return s_sb[:, base:base + 256].rearrange(
                    "p (kt2 j) -> p kt2 j", kt2=2)

            dr_rot = [0]

            def dcopy(out, in_):
                """psum->SBUF drain copy, alternating DVE / ACT."""
                dr_rot[0] ^= 1
                if dr_rot[0]:
                    nc.vector.tensor_copy(out=out, in_=in_)
                else:
                    nc.scalar.copy(out=out, in_=in_)

            def x0sl(t, b):
                lo = nw + (t * bs + b) * 128
                return xw_sb[:, lo:lo + 128]

            # ---- ph1a: slots 0,1 (2u2/16, 2u4/16); 8 b per 2-bank mega ----
            def ph1a(pp, t):
                ps = pp.tile([128, 1024], F32, name="ps_a", tag="pp")
                for i in range(bs):
                    nc.tensor.matmul(
                        ps[:, i * 128:(i + 1) * 128],
                        lhsT=x0sl(t, i),
                        rhs=w_sb[:, 0:128],
                        start=True, stop=True,
                    )
                src = ps.rearrange("p (b8 mi2 o) -> p b8 mi2 o", b8=bs, mi2=2)
                dcopy(ub_view(t, (0, 1), 0, bs), src)

            # ---- ph1b: slots 2,3 + v0; 2-bank megatile, 4 b at 256 pitch ----
            def ph1b(pb, t, bq):
                ps = pb.tile([128, 1024], F32, name="ps_b", tag="pp")
                for i in range(4):
                    nc.tensor.matmul(
                        ps[:, i * 256:i * 256 + 192],
                        lhsT=x0sl(t, bq * 4 + i),
                        rhs=w_sb[:, 128:320],
                        start=True, stop=True,
                    )
                src = ps.rearrange("p (b4 mi4 o) -> p b4 mi4 o", b4=4, mi4=4)
                dcopy(ub_view(t, (2, 3), bq * 4, 4), src[:, :, 0:2, :])
                v0dst = V0[t].rearrange("p (b8 o) -> p b8 o", b8=bs)[
                    :, bq * 4:bq * 4 + 4, :]
                dcopy(v0dst, src[:, :, 2, :])

            # ---- hop: acc[t] = (256 s) @ slot_src over all k; c = 16u + acc
            def hop_tile(pacc, s_sb, t, src_slot, dst_slot):
                ps = pacc.tile([128, obs], F32, name="ps_acc", tag="acc")
                for ktp in range(nt // 2):
                    nc.tensor.matmul(
                        ps[:, :],
                        lhsT=strip(s_sb, t, ktp),
                        rhs=upair(ktp, src_slot),
                        start=(ktp == 0),
                        stop=(ktp == nt // 2 - 1),
                        perf_mode=mybir.MatmulPerfMode.DoubleRow,
                    )
                d = uslot(t, dst_slot)
                nc.vector.tensor_add(d, d, ps[:, :])

            # ---- final: V[t] = v0 + (s0@c0 + s1@c1)/(SCALE*USCALE); dma out
            def final_tile(pacc, t):
                ps = pacc.tile([128, obs], F32, name="ps_acc", tag="acc")
                for g, (s_sb, sl) in enumerate([(s0_sb, 2), (s1_sb, 3)]):
                    for ktp in range(nt // 2):
                        nc.tensor.matmul(
                            ps[:, :],
                            lhsT=strip(s_sb, t, ktp),
                            rhs=upair(ktp, sl),
                            start=(g == 0 and ktp == 0),
                            stop=(g == 1 and ktp == nt // 2 - 1),
                            perf_mode=mybir.MatmulPerfMode.DoubleRow,
                        )
                nc.vector.scalar_tensor_tensor(
                    out=V[t][:, :], in0=ps[:, :], scalar=1.0 / (SCALE * USCALE),
                    op0=mybir.AluOpType.mult,
                    in1=V0[t][:, :], op1=mybir.AluOpType.add)
                nc.sync.dma_start(
                    out=out[t * 128:(t + 1) * 128, :], in_=V[t][:, :]
                )

            for _rep in range(repeat):
                with (
                    tc.tile_pool(name="pp", bufs=3, space="PSUM") as pp,
                    tc.tile_pool(name="pacc", bufs=2, space="PSUM") as pacc,
                ):
                    for t in range(nt):
                        ph1a(pp, t)
                    # ph1b megas for tile t precede hop1's tile t so the
                    # hop1 drain (c0 = 16u1 + acc) finds slots 2,3 ready;
                    # hop1 matmuls chase the ph1a drain wave + s0 DMA.
                    for t in range(nt):
                        ph1b(pp, t, 0)
                        ph1b(pp, t, 1)
                        hop_tile(pacc, s0_sb, t, 0, 2)
                    for t in range(nt):
                        hop_tile(pacc, s1_sb, t, 1, 3)
                    for t in range(nt):
                        final_tile(pacc, t)
    return _legalize_waits(nc) if legalize else nc


_NC_CACHE = {}


def _get_nc():
    if "nc" not in _NC_CACHE:
        _NC_CACHE["nc"] = build_bass()
    return _NC_CACHE["nc"]


def make_inputs(support0, support1, inputs, state, weight):
    """Host-side layout prep -> per-core in_maps (shared replicated arrays)."""
    xs = np.concatenate(
        [
            np.asarray(inputs, np.float32).reshape(B, N, F // 2),
            np.asarray(state, np.float32).reshape(B, N, F // 2),
        ],
        axis=2,
    )  # [B, N, F]

    w = np.asarray(weight, np.float32).reshape(F, M5, O)
    wv0 = w[:, 0] - w[:, 2] - w[:, 4]
    wcat = np.concatenate(
        [2.0 * w[:, 2] / USCALE, 2.0 * w[:, 4] / USCALE,
         USCALE * w[:, 1], USCALE * w[:, 3], wv0], axis=1
    ).astype(NPBF16)  # [128, 320]

    def strip_img(s):
        # fp8 DoubleRow pair layout: [t, p, ktp*256 + kt2*128 + j]
        #   = fp8(SCALE * s[t*128+j, (ktp*2+kt2)*128 + p])
        r = (SCALE * np.asarray(s, np.float32)).astype(NPFP8)
        r = r.reshape(NT, 128, NT, 128).transpose(0, 3, 2, 1)  # [t, p, kt, j]
        return np.ascontiguousarray(r.reshape(NT, 128, N))

    s0i, s1i = strip_img(support0), strip_img(support1)

    in_maps = []
    for c in range(NCORES):
        shard = xs[c * BS:(c + 1) * BS]                # [8b, N, F]
        # t-major SBUF image: x0t[f, t*BS*128 + b*128 + j] = shard[b, t*128+j, f]
        x0t = np.ascontiguousarray(
            shard.reshape(BS, NT, 128, F).transpose(3, 1, 0, 2).reshape(F, BS * N)
        ).astype(NPBF16)
        in_maps.append({"x0t": x0t, "wcat": wcat, "s0t": s0i, "s1t": s1i})
    return in_maps


def postprocess(results, biases):
    full = np.empty((B, N, O), np.float32)
    for c, r in enumerate(results):
        full[c * BS:(c + 1) * BS] = (
            r["out"].astype(np.float32).reshape(N, BS, O).transpose(1, 0, 2)
        )
    full += np.asarray(biases, np.float32)[None, None, :]
    return full.reshape(B, N * O)


def kernel(support0, support1, inputs, state, weight, biases, output_size=None,
           **run_kwargs):
    nc = _get_nc()
    in_maps = make_inputs(support0, support1, inputs, state, weight)
    res = run_bass_kernel_spmd(nc, in_maps, core_ids=list(range(NCORES)),
                               **run_kwargs)
    out = postprocess(res.results, biases)
    if run_kwargs.get("trace"):
        return out, res
    return out


# revision 19
# speedup vs baseline: 32.6173x; 1.0832x over previous
"""DiffusionGraphConv Trainium2 kernel (8-core SPMD, data-parallel over batch).

Math (halves big-matmul FLOPs vs the reference order):
  reference: out[b,n,o] = sum_{f,m} mats_m[n,f,b] * W[f*5+m, o]
  with mats = [x0, s0 x0, 2 s0^2 x0 - x0, s1 x0, 2 s1^2 x0 - x0].
  Projection (F=128 -> O=64) commutes with node-space diffusion:
    u_m = proj(x0, W_m)                       # [N, O, B] each, cheap
    v0 = proj(x0, W0-W2-W4)
    c0 = 16*u1 + (256 s0) @ (2u2/16);  c1 analogous (fp8-safe scaling)
    out = v0 + (s0 @ c0 + s1 @ c1) / (256*16)

Schedule (cost-model-driven; TimelineSim is the tuning target):
  - All input DMAs issued up front on the SP queue in arrival-deadline order
    (wcat, x0 x8 chunks, s0 halves, s1 halves). DMA transfers serialize on
    the one DMA_ENGINES resource, so order == arrival time.
  - Both supports live fully resident in SBUF (fp8 strips, 32KB/partition
    each) — loaded once, read by hops 1,2 and the final merged hop.
  - ph1a: projections for slots {2u2, 2u4}; 4 batches packed per PSUM bank
    so each bank drains with ONE strided copy (drain overhead dominates the
    DVE/ACT budget otherwise).
  - ph1b: projections for {16u1, 16u3, v0} in 2-bank PSUM megatiles (4 b
    each); emitted after ph1a and interleaved into hop1 — fills the PE idle
    while the s0 strip DMA lands.
  - hop1/hop2: per node-tile, 8 DoubleRow fp8 matmuls accumulate
    (256 s)@(2u/16); drain = in-place fp8 add onto the u1/u3 slot -> c0/c1.
  - final: per node-tile one 16-matmul group (s0@c0 + s1@c1), drained with
    scalar_tensor_tensor into bf16 V, DMA'd out per tile (bf16 out, host
    upcasts and adds biases).
  - All psum drains alternate DVE/ACT (Pool has no PSUM port).

Env quirks handled here: walrus accepts <=1 sync-wait per instruction
(_legalize_waits hoists extras onto EventSemaphore carriers; simulators need
legalize=False); repeat=N re-runs the idempotent pipeline for wall-clock
differencing since this axon terminal has no NTFF profiling.
"""

import sys

if "/opt/trn_rl_repo" not in sys.path:
    sys.path.insert(0, "/opt/trn_rl_repo")

import numpy as np
import ml_dtypes

import concourse.bass as bass
import concourse.mybir as mybir
from concourse.tile import TileContext
from concourse.bass_utils import run_bass_kernel_spmd

BF16 = mybir.dt.bfloat16
FP8 = mybir.dt.float8e4
NPFP8 = ml_dtypes.float8_e4m3
SCALE = 256.0
USCALE = 16.0
F32 = mybir.dt.float32
NPBF16 = ml_dtypes.bfloat16

N = 2048          # graph nodes
F = 128           # input_size (64 input + 64 hidden)
B = 64            # global batch
NCORES = 8
BS = B // NCORES  # 8 batches per core
O = 64            # output features
NT = N // 128     # 16 node tiles
M5 = 5            # diffusion matrices


def _legalize_waits(nc, max_waits=1):
    """Walrus in this env encodes at most one sync-wait per instruction.

    Tile's sem assignment can emit 2-3 waits on one instruction; hoist the
    excess onto standalone EventSemaphore carriers (same engine, inserted
    just before), which the sequencer executes in order — semantics are
    identical, encoding is legal."""
    f = nc.m.functions[0]
    for blk in f.blocks:
        new_insts = []
        changed = False
        for inst in blk.instructions:
            si = inst.sync_info
            waits = list(si.on_wait) if si is not None else []
            if len(waits) > max_waits:
                for i, w in enumerate(waits[:-max_waits]):
                    ev = mybir.InstEventSemaphore(
                        name=f"{inst.name}-wsplit{i}",
                        engine=inst.engine,
                        ins=[],
                        outs=[],
                        sync_info=mybir.SyncInfo(on_wait=[w], on_update=[]),
                    )
                    new_insts.append(ev)
                inst.sync_info = mybir.SyncInfo(
                    on_wait=waits[-max_waits:], on_update=list(si.on_update)
                )
                changed = True
            new_insts.append(inst)
        if changed:
            blk.instructions = new_insts
    return nc


def build_bass(n=N, bs=BS, o=O, legalize=True, repeat=1, lead=8,
               pp_bufs=3, pacc_bufs=2):
    """Build the per-core SPMD Bass program."""
    nt = n // 128
    obs = bs * o        # 512: width of diffusion operands
    nc = bass.Bass()
    # xw: [wcat | x0] fused so one DMA unblocks the first projection.
    # wcat: [2W2/16 | 2W4/16 | 16W1 | 16W3 | W0-W2-W4]  (320 cols)
    nw = M5 * o
    xw = nc.dram_tensor("xw", [F, nw + bs * n], BF16, kind="ExternalInput")
    s0t = nc.dram_tensor("s0t", [nt, 128, n], FP8, kind="ExternalInput")
    s1t = nc.dram_tensor("s1t", [nt, 128, n], FP8, kind="ExternalInput")
    out = nc.dram_tensor("out", [n, obs], BF16, kind="ExternalOutput")

    with TileContext(nc) as tc:
        with tc.tile_pool(name="persist", bufs=1) as persist:
            # ---- all input DMAs up front, in deadline order ----
            xw_sb = persist.tile([F, nw + bs * n], BF16, name="xw_sb")
            w_sb = xw_sb[:, 0:nw]
            tb = bs * 128      # columns per node-tile of x0
            # t0 chunk first: ph1a's first Ldweights needs only x0;
            # the matmul's rhs (w) lands one small transfer later.
            spans = [(nw, nw + tb), (0, nw)]
            cuts = [nw + t * tb for t in (2, 3, 4)] + [
                nw + t * tb for t in range(6, nt + 1, 2)]
            spans += list(zip([nw + tb] + cuts[:-1], cuts))
            for lo, hi in spans:
                nc.sync.dma_start(out=xw_sb[:, lo:hi], in_=xw[:, lo:hi])
            s0_sb = persist.tile([128, nt * n], FP8, name="s0_sb")
            s1_sb = persist.tile([128, nt * n], FP8, name="s1_sb")
            for s_sb, sdram in ((s0_sb, s0t), (s1_sb, s1t)):
                for h in range(2):
                    half = nt // 2
                    nc.sync.dma_start(
                        out=s_sb[:, h * half * n:(h + 1) * half * n].rearrange(
                            "p (t j) -> p t j", t=half),
                        in_=sdram.rearrange("t p j -> p t j")[
                            :, h * half:(h + 1) * half, :],
                    )

            # U[tp]: [128, 4 slots * 2 kt * obs] fp8.
            # slots: 0 = 2u2/16, 1 = 2u4/16, 2 = 16u1 -> c0, 3 = 16u3 -> c1.
            U = [
                persist.tile([128, 4 * 2 * obs], FP8, name=f"u{tp}", tag=f"u{tp}")
                for tp in range(nt // 2)
            ]
            V = [
                persist.tile([128, obs], BF16, name=f"v{t}", tag=f"v{t}")
                for t in range(nt)
            ]

            def upair(tp, slot):
                """[128, 2, obs] DoubleRow moving view: k-tile pair of a slot."""
                return U[tp].rearrange(
                    "p (mi4 kt2 c) -> p mi4 kt2 c", mi4=4, kt2=2)[:, slot, :, :]

            def uslot(t, slot):
                """[128, obs] contiguous view of a slot for node-tile t."""
                base = slot * 2 * obs + (t % 2) * obs
                return U[t // 2][:, base:base + obs]

            def ub_view(t, slots, b0, nb):
                """[128, nb, len(slots), o] strided write view of U."""
                v = U[t // 2].rearrange(
                    "p (mi4 kt2 b8 o) -> p b8 mi4 kt2 o", mi4=4, kt2=2, b8=bs
                )[:, b0:b0 + nb, slots[0]:slots[-1] + 1, t % 2, :]
                return v

            def strip(s_sb, t, ktp):
                """[128, 2, 128] DoubleRow stationary view of support strip."""
                base = t * n + ktp * 256
                return s_sb[:, base:base + 256].rearrange(
                    "p (kt2 j) -> p kt2 j", kt2=2)

            dr_rot = [0]

            def dcopy(out, in_):
                """psum->SBUF drain copy, alternating DVE / ACT."""
                dr_rot[0] ^= 1
                if dr_rot[0]:
                    nc.vector.tensor_copy(out=out, in_=in_)
                else:
                    nc.scalar.copy(out=out, in_=in_)

            def x0sl(t, b):
                lo = nw + (t * bs + b) * 128
                return xw_sb[:, lo:lo + 128]

            # ---- ph1a: slots 0,1 (2u2/16, 2u4/16); 8 b per 2-bank mega ----
            def ph1a(pp, t):
                ps = pp.tile([128, 1024], F32, name="ps_a", tag="pp")
                for i in range(bs):
                    nc.tensor.matmul(
                        ps[:, i * 128:(i + 1) * 128],
                        lhsT=x0sl(t, i),
                        rhs=w_sb[:, 0:128],
                        start=True, stop=True,
                    )
                src = ps.rearrange("p (b8 mi2 o) -> p b8 mi2 o", b8=bs, mi2=2)
                h = bs // 2
                dcopy(ub_view(t, (0, 1), 0, h), src[:, 0:h])
                dcopy(ub_view(t, (0, 1), h, h), src[:, h:bs])

            # ---- ph1b: slots 2,3 (16u1, 16u3); 8 b per 2-bank mega ----
            # (v0 is recomputed during the final phase instead of being
            # drained here — keeps the drain-heavy first half PE-bound)
            def ph1b(pb, t):
                ps = pb.tile([128, 1024], F32, name="ps_b", tag="pp")
                for i in range(bs):
                    nc.tensor.matmul(
                        ps[:, i * 128:(i + 1) * 128],
                        lhsT=x0sl(t, i),
                        rhs=w_sb[:, 128:256],
                        start=True, stop=True,
                    )
                src = ps.rearrange("p (b8 mi2 o) -> p b8 mi2 o", b8=bs, mi2=2)
                h = bs // 2
                dcopy(ub_view(t, (2, 3), 0, h), src[:, 0:h])
                dcopy(ub_view(t, (2, 3), h, h), src[:, h:bs])

            # ---- hop: acc[t] = (256 s) @ slot_src over all k; c = 16u + acc
            def hop_tile(pacc, s_sb, t, src_slot, dst_slot):
                ps = pacc.tile([128, obs], F32, name="ps_acc", tag="acc")
                for ktp in range(nt // 2):
                    nc.tensor.matmul(
                        ps[:, :],
                        lhsT=strip(s_sb, t, ktp),
                        rhs=upair(ktp, src_slot),
                        start=(ktp == 0),
                        stop=(ktp == nt // 2 - 1),
                        perf_mode=mybir.MatmulPerfMode.DoubleRow,
                    )
                d = uslot(t, dst_slot)
                nc.vector.tensor_add(d, d, ps[:, :])

            # ---- final: V[t] = v0 + (s0@c0 + s1@c1)/(SCALE*USCALE); dma out
            def final_tile(pacc, pv, vtmp, t):
                psv = pv.tile([128, obs], F32, name="ps_v0", tag="pv")
                for i in range(bs):
                    nc.tensor.matmul(
                        psv[:, i * o:(i + 1) * o],
                        lhsT=x0sl(t, i),
                        rhs=w_sb[:, 256:320],
                        start=True, stop=True,
                    )
                # engines can read only ONE psum operand per instruction:
                # stage v0 through SBUF on the (otherwise idle) ACT engine
                vt = vtmp.tile([128, obs], BF16, name="vt", tag="vt")
                nc.scalar.copy(out=vt[:, :], in_=psv[:, :])
                ps = pacc.tile([128, obs], F32, name="ps_acc", tag="acc")
                for g, (s_sb, sl) in enumerate([(s0_sb, 2), (s1_sb, 3)]):
                    for ktp in range(nt // 2):
                        nc.tensor.matmul(
                            ps[:, :],
                            lhsT=strip(s_sb, t, ktp),
                            rhs=upair(ktp, sl),
                            start=(g == 0 and ktp == 0),
                            stop=(g == 1 and ktp == nt // 2 - 1),
                            perf_mode=mybir.MatmulPerfMode.DoubleRow,
                        )
                nc.vector.scalar_tensor_tensor(
                    out=V[t][:, :], in0=ps[:, :], scalar=1.0 / (SCALE * USCALE),
                    op0=mybir.AluOpType.mult,
                    in1=vt[:, :], op1=mybir.AluOpType.add)
                nc.sync.dma_start(
                    out=out[t * 128:(t + 1) * 128, :], in_=V[t][:, :]
                )

            for _rep in range(repeat):
                with tc.tile_pool(name="pa", bufs=4, space="PSUM") as pa:
                    for t in range(nt):
                        ph1a(pa, t)
                with tc.tile_pool(name="pacc", bufs=pacc_bufs,
                                  space="PSUM") as pacc:
                    with tc.tile_pool(name="pp", bufs=pp_bufs,
                                      space="PSUM") as pp:
                        # ph1b lead fills PE while the s0 strip DMA lands
                        # (PE issue is in-order: only already-emitted megas
                        # can run during the wait).
                        mi = 0
                        for _ in range(min(lead, nt)):
                            ph1b(pp, mi); mi += 1
                        for t in range(nt):
                            if mi < nt:
                                ph1b(pp, mi); mi += 1
                            hop_tile(pacc, s0_sb, t, 0, 2)
                        for t in range(nt):
                            hop_tile(pacc, s1_sb, t, 1, 3)
                    with (
                        tc.tile_pool(name="pv", bufs=2, space="PSUM") as pv,
                        tc.tile_pool(name="vtmp", bufs=3) as vtmp,
                    ):
                        for t in range(nt):
                            final_tile(pacc, pv, vtmp, t)
    return _legalize_waits(nc) if legalize else nc


_NC_CACHE = {}


def _get_nc():
    if "nc" not in _NC_CACHE:
        _NC_CACHE["nc"] = build_bass()
    return _NC_CACHE["nc"]


def make_inputs(support0, support1, inputs, state, weight):
    """Host-side layout prep -> per-core in_maps (shared replicated arrays)."""
    xs = np.concatenate(
        [
            np.asarray(inputs, np.float32).reshape(B, N, F // 2),
            np.asarray(state, np.float32).reshape(B, N, F // 2),
        ],
        axis=2,
    )  # [B, N, F]

    w = np.asarray(weight, np.float32).reshape(F, M5, O)
    wv0 = w[:, 0] - w[:, 2] - w[:, 4]
    wcat = np.concatenate(
        [2.0 * w[:, 2] / USCALE, 2.0 * w[:, 4] / USCALE,
         USCALE * w[:, 1], USCALE * w[:, 3], wv0], axis=1
    ).astype(NPBF16)  # [128, 320]

    def strip_img(s):
        # fp8 DoubleRow pair layout: [t, p, ktp*256 + kt2*128 + j]
        #   = fp8(SCALE * s[t*128+j, (ktp*2+kt2)*128 + p])
        r = (SCALE * np.asarray(s, np.float32)).astype(NPFP8)
        r = r.reshape(NT, 128, NT, 128).transpose(0, 3, 2, 1)  # [t, p, kt, j]
        return np.ascontiguousarray(r.reshape(NT, 128, N))

    s0i, s1i = strip_img(support0), strip_img(support1)

    in_maps = []
    for c in range(NCORES):
        shard = xs[c * BS:(c + 1) * BS]                # [8b, N, F]
        # t-major image: x0t[f, t*BS*128 + b*128 + j] = shard[b, t*128+j, f];
        # fused as xw = [wcat | x0t] so one DMA feeds the first projection.
        x0t = shard.reshape(BS, NT, 128, F).transpose(3, 1, 0, 2).reshape(
            F, BS * N).astype(NPBF16)
        xw = np.ascontiguousarray(np.concatenate([wcat, x0t], axis=1))
        in_maps.append({"xw": xw, "s0t": s0i, "s1t": s1i})
    return in_maps


def postprocess(results, biases):
    full = np.empty((B, N, O), np.float32)
    for c, r in enumerate(results):
        full[c * BS:(c + 1) * BS] = (
            r["out"].astype(np.float32).reshape(N, BS, O).transpose(1, 0, 2)
        )
    full += np.asarray(biases, np.float32)[None, None, :]
    return full.reshape(B, N * O)


def kernel(support0, support1, inputs, state, weight, biases, output_size=None,
           **run_kwargs):
    nc = _get_nc()
    in_maps = make_inputs(support0, support1, inputs, state, weight)
    res = run_bass_kernel_spmd(nc, in_maps, core_ids=list(range(NCORES)),
                               **run_kwargs)
    out = postprocess(res.results, biases)
    if run_kwargs.get("trace"):
        return out, res
    return out


# revision 27
# speedup vs baseline: 33.8894x; 1.0390x over previous
"""DiffusionGraphConv Trainium2 kernel (8-core SPMD, data-parallel over batch).

Math (halves big-matmul FLOPs vs the reference order):
  reference: out[b,n,o] = sum_{f,m} mats_m[n,f,b] * W[f*5+m, o]
  with mats = [x0, s0 x0, 2 s0^2 x0 - x0, s1 x0, 2 s1^2 x0 - x0].
  Projection (F=128 -> O=64) commutes with node-space diffusion:
    u_m = proj(x0, W_m)                       # [N, O, B] each, cheap
    v0 = proj(x0, W0-W2-W4)
    c0 = 16*u1 + (256 s0) @ (2u2/16);  c1 analogous (fp8-safe scaling)
    out = v0 + (s0 @ c0 + s1 @ c1) / (256*16)

Schedule (cost-model-driven; TimelineSim is the tuning target):
  - All input DMAs issued up front on the SP queue in arrival-deadline order
    (wcat, x0 x8 chunks, s0 halves, s1 halves). DMA transfers serialize on
    the one DMA_ENGINES resource, so order == arrival time.
  - Both supports live fully resident in SBUF (fp8 strips, 32KB/partition
    each) — loaded once, read by hops 1,2 and the final merged hop.
  - ph1a: projections for slots {2u2, 2u4}; 4 batches packed per PSUM bank
    so each bank drains with ONE strided copy (drain overhead dominates the
    DVE/ACT budget otherwise).
  - ph1b: projections for {16u1, 16u3, v0} in 2-bank PSUM megatiles (4 b
    each); emitted after ph1a and interleaved into hop1 — fills the PE idle
    while the s0 strip DMA lands.
  - hop1/hop2: per node-tile, 8 DoubleRow fp8 matmuls accumulate
    (256 s)@(2u/16); drain = in-place fp8 add onto the u1/u3 slot -> c0/c1.
  - final: per node-tile one 16-matmul group (s0@c0 + s1@c1), drained with
    scalar_tensor_tensor into bf16 V, DMA'd out per tile (bf16 out, host
    upcasts and adds biases).
  - All psum drains alternate DVE/ACT (Pool has no PSUM port).

Env quirks handled here: walrus accepts <=1 sync-wait per instruction
(_legalize_waits hoists extras onto EventSemaphore carriers; simulators need
legalize=False); repeat=N re-runs the idempotent pipeline for wall-clock
differencing since this axon terminal has no NTFF profiling.
"""

import sys

if "/opt/trn_rl_repo" not in sys.path:
    sys.path.insert(0, "/opt/trn_rl_repo")

import numpy as np
import ml_dtypes

import concourse.bass as bass
import concourse.mybir as mybir
from concourse.tile import TileContext
from concourse.bass_utils import run_bass_kernel_spmd

BF16 = mybir.dt.bfloat16
FP8 = mybir.dt.float8e4
NPFP8 = ml_dtypes.float8_e4m3
SCALE = 256.0
USCALE = 16.0
F32 = mybir.dt.float32
NPBF16 = ml_dtypes.bfloat16

N = 2048          # graph nodes
F = 128           # input_size (64 input + 64 hidden)
B = 64            # global batch
NCORES = 8
BS = B // NCORES  # 8 batches per core
O = 64            # output features
NT = N // 128     # 16 node tiles
M5 = 5            # diffusion matrices


def _legalize_waits(nc, max_waits=1):
    """Walrus in this env encodes at most one sync-wait per instruction.

    Tile's sem assignment can emit 2-3 waits on one instruction; hoist the
    excess onto standalone EventSemaphore carriers (same engine, inserted
    just before), which the sequencer executes in order — semantics are
    identical, encoding is legal."""
    f = nc.m.functions[0]
    for blk in f.blocks:
        new_insts = []
        changed = False
        for inst in blk.instructions:
            si = inst.sync_info
            waits = list(si.on_wait) if si is not None else []
            if len(waits) > max_waits:
                for i, w in enumerate(waits[:-max_waits]):
                    ev = mybir.InstEventSemaphore(
                        name=f"{inst.name}-wsplit{i}",
                        engine=inst.engine,
                        ins=[],
                        outs=[],
                        sync_info=mybir.SyncInfo(on_wait=[w], on_update=[]),
                    )
                    new_insts.append(ev)
                inst.sync_info = mybir.SyncInfo(
                    on_wait=waits[-max_waits:], on_update=list(si.on_update)
                )
                changed = True
            new_insts.append(inst)
        if changed:
            blk.instructions = new_insts
    return nc


def build_bass(n=N, bs=BS, o=O, legalize=True, repeat=1, lead=2,
               pp_bufs=3, pacc_bufs=2):
    """Build the per-core SPMD Bass program."""
    nt = n // 128
    obs = bs * o        # 512: width of diffusion operands
    nc = bass.Bass()
    # xw: [wcat | x0] fused so one DMA unblocks the first projection.
    # wcat: [2W2/16 | 2W4/16 | 16W1 | 16W3 | W0-W2-W4]  (320 cols)
    nw = M5 * o
    xw = nc.dram_tensor("xw", [F, nw + bs * n], BF16, kind="ExternalInput")
    s0t = nc.dram_tensor("s0t", [nt, 128, n], FP8, kind="ExternalInput")
    s1t = nc.dram_tensor("s1t", [nt, 128, n], FP8, kind="ExternalInput")
    out = nc.dram_tensor("out", [n, obs], BF16, kind="ExternalOutput")

    with TileContext(nc) as tc:
        with tc.tile_pool(name="persist", bufs=1) as persist:
            # ---- all input DMAs up front, in deadline order ----
            xw_sb = persist.tile([F, nw + bs * n], BF16, name="xw_sb")
            w_sb = xw_sb[:, 0:nw]
            tb = bs * 128      # columns per node-tile of x0
            # t0 chunk first: ph1a's first Ldweights needs only x0;
            # the matmul's rhs (w) lands one small transfer later.
            spans = [(nw, nw + tb), (0, nw)]
            cuts = [nw + t * tb for t in range(2, nt + 1)]
            spans += list(zip([nw + tb] + cuts[:-1], cuts))
            for lo, hi in spans:
                nc.sync.dma_start(out=xw_sb[:, lo:hi], in_=xw[:, lo:hi])
            s0_sb = persist.tile([128, nt * n], FP8, name="s0_sb")
            s1_sb = persist.tile([128, nt * n], FP8, name="s1_sb")
            for s_sb, sdram in ((s0_sb, s0t), (s1_sb, s1t)):
                for h in range(4):
                    q = nt // 4
                    nc.sync.dma_start(
                        out=s_sb[:, h * q * n:(h + 1) * q * n].rearrange(
                            "p (t j) -> p t j", t=q),
                        in_=sdram.rearrange("t p j -> p t j")[
                            :, h * q:(h + 1) * q, :],
                    )

            # U[tp]: [128, 4 slots * 2 kt * obs] fp8.
            # slots: 0 = 2u2/16, 1 = 2u4/16, 2 = 16u1 -> c0, 3 = 16u3 -> c1.
            U = [
                persist.tile([128, 4 * 2 * obs], FP8, name=f"u{tp}", tag=f"u{tp}")
                for tp in range(nt // 2)
            ]
            V = [
                persist.tile([128, obs], BF16, name=f"v{t}", tag=f"v{t}")
                for t in range(nt)
            ]

            def upair(tp, slot):
                """[128, 2, obs] DoubleRow moving view: k-tile pair of a slot."""
                return U[tp].rearrange(
                    "p (mi4 kt2 c) -> p mi4 kt2 c", mi4=4, kt2=2)[:, slot, :, :]

            def uslot(t, slot):
                """[128, obs] contiguous view of a slot for node-tile t."""
                base = slot * 2 * obs + (t % 2) * obs
                return U[t // 2][:, base:base + obs]

            def ub_view(t, slots, b0, nb):
                """[128, nb, len(slots), o] strided write view of U."""
                v = U[t // 2].rearrange(
                    "p (mi4 kt2 b8 o) -> p b8 mi4 kt2 o", mi4=4, kt2=2, b8=bs
                )[:, b0:b0 + nb, slots[0]:slots[-1] + 1, t % 2, :]
                return v

            def strip(s_sb, t, ktp):
                """[128, 2, 128] DoubleRow stationary view of support strip."""
                base = t * n + ktp * 256
                return s_sb[:, base:base + 256].rearrange(
                    "p (kt2 j) -> p kt2 j", kt2=2)

            dr_rot = [0]

            def dcopy(out, in_):
                """psum->SBUF drain copy, alternating DVE / ACT."""
                dr_rot[0] ^= 1
                if dr_rot[0]:
                    nc.vector.tensor_copy(out=out, in_=in_)
                else:
                    nc.scalar.copy(out=out, in_=in_)

            def x0sl(t, b):
                lo = nw + (t * bs + b) * 128
                return xw_sb[:, lo:lo + 128]

            # ---- ph1a: slots 0,1 (2u2/16, 2u4/16); 8 b per 2-bank mega.
            # The tail megas split their drain into parallel halves so the
            # phase's last drain latency (which gates ph1b/hop1) is halved.
            def ph1a(pa, t):
                ps = pa.tile([128, 1024], F32, name="ps_a", tag="pa")
                for i in range(bs):
                    nc.tensor.matmul(
                        ps[:, i * 128:(i + 1) * 128],
                        lhsT=x0sl(t, i),
                        rhs=w_sb[:, 0:128],
                        start=True, stop=True,
                    )
                src = ps.rearrange("p (b8 mi2 o) -> p b8 mi2 o", b8=bs, mi2=2)
                dcopy(ub_view(t, (0, 1), 0, bs), src)

            # ---- ph1b: slots 2,3 (16u1, 16u3); 8 b per 2-bank mega ----
            # (v0 is recomputed during the final phase instead of being
            # drained here — keeps the drain-heavy first half PE-bound)
            def ph1b(pb, t):
                ps = pb.tile([128, 1024], F32, name="ps_b", tag="pp")
                for i in range(bs):
                    nc.tensor.matmul(
                        ps[:, i * 128:(i + 1) * 128],
                        lhsT=x0sl(t, i),
                        rhs=w_sb[:, 128:256],
                        start=True, stop=True,
                    )
                src = ps.rearrange("p (b8 mi2 o) -> p b8 mi2 o", b8=bs, mi2=2)
                dcopy(ub_view(t, (2, 3), 0, bs), src)

            # ---- hop: acc[t] = (256 s) @ slot_src over all k; c = 16u + acc
            def hop_tile(pacc, s_sb, t, src_slot, dst_slot):
                ps = pacc.tile([128, 1024], F32, name="ps_acc", tag="pp")[:, 0:obs]
                for ktp in range(nt // 2):
                    nc.tensor.matmul(
                        ps[:, :],
                        lhsT=strip(s_sb, t, ktp),
                        rhs=upair(ktp, src_slot),
                        start=(ktp == 0),
                        stop=(ktp == nt // 2 - 1),
                        perf_mode=mybir.MatmulPerfMode.DoubleRow,
                    )
                d = uslot(t, dst_slot)
                nc.vector.tensor_add(d, d, ps[:, :])

            # ---- final: V[t] = v0 + (s0@c0 + s1@c1)/(SCALE*USCALE); dma out
            def final_tile(pf, pv, vtmp, t):
                psv = pv.tile([128, obs], F32, name="ps_v0", tag="pv")
                for i in range(bs):
                    nc.tensor.matmul(
                        psv[:, i * o:(i + 1) * o],
                        lhsT=x0sl(t, i),
                        rhs=w_sb[:, 256:320],
                        start=True, stop=True,
                    )
                # engines can read only ONE psum operand per instruction:
                # stage v0 through SBUF on the (otherwise idle) ACT engine
                vt = vtmp.tile([128, obs], BF16, name="vt", tag="vt")
                nc.scalar.copy(out=vt[:, :], in_=psv[:, :])
                ps = pf.tile([128, obs], F32, name="ps_acc", tag="acc")
                for g, (s_sb, sl) in enumerate([(s0_sb, 2), (s1_sb, 3)]):
                    for ktp in range(nt // 2):
                        nc.tensor.matmul(
                            ps[:, :],
                            lhsT=strip(s_sb, t, ktp),
                            rhs=upair(ktp, sl),
                            start=(g == 0 and ktp == 0),
                            stop=(g == 1 and ktp == nt // 2 - 1),
                            perf_mode=mybir.MatmulPerfMode.DoubleRow,
                        )
                halves = 2 if t == nt - 1 else 1
                hw_ = obs // halves
                for hh in range(halves):
                    sl = slice(hh * hw_, (hh + 1) * hw_)
                    nc.vector.scalar_tensor_tensor(
                        out=V[t][:, sl], in0=ps[:, sl],
                        scalar=1.0 / (SCALE * USCALE),
                        op0=mybir.AluOpType.mult,
                        in1=vt[:, sl], op1=mybir.AluOpType.add)
                    nc.sync.dma_start(
                        out=out[t * 128:(t + 1) * 128, sl], in_=V[t][:, sl]
                    )

            for _rep in range(repeat):
                with tc.tile_pool(name="pa", bufs=4, space="PSUM") as pa:
                    for t in range(nt):
                        ph1a(pa, t)
                with tc.tile_pool(name="pz", bufs=4, space="PSUM") as pz:
                    # ph1b lead fills PE while the s0 strip DMA lands
                    # (PE issue is in-order: only already-emitted megas
                    # can run during the wait).
                    mi = 0
                    for _ in range(min(lead, nt)):
                        ph1b(pz, mi); mi += 1
                    for t in range(nt):
                        if mi < nt:
                            ph1b(pz, mi); mi += 1
                        hop_tile(pz, s0_sb, t, 0, 2)
                    for t in range(nt):
                        hop_tile(pz, s1_sb, t, 1, 3)
                with (
                    tc.tile_pool(name="pf", bufs=3, space="PSUM") as pf,
                    tc.tile_pool(name="pv", bufs=2, space="PSUM") as pv,
                    tc.tile_pool(name="vtmp", bufs=3) as vtmp,
                ):
                    for t in range(nt):
                        final_tile(pf, pv, vtmp, t)
    return _legalize_waits(nc) if legalize else nc


_NC_CACHE = {}


def _get_nc():
    if "nc" not in _NC_CACHE:
        _NC_CACHE["nc"] = build_bass()
    return _NC_CACHE["nc"]


def make_inputs(support0, support1, inputs, state, weight):
    """Host-side layout prep -> per-core in_maps (shared replicated arrays)."""
    xs = np.concatenate(
        [
            np.asarray(inputs, np.float32).reshape(B, N, F // 2),
            np.asarray(state, np.float32).reshape(B, N, F // 2),
        ],
        axis=2,
    )  # [B, N, F]

    w = np.asarray(weight, np.float32).reshape(F, M5, O)
    wv0 = w[:, 0] - w[:, 2] - w[:, 4]
    wcat = np.concatenate(
        [2.0 * w[:, 2] / USCALE, 2.0 * w[:, 4] / USCALE,
         USCALE * w[:, 1], USCALE * w[:, 3], wv0], axis=1
    ).astype(NPBF16)  # [128, 320]

    def strip_img(s):
        # fp8 DoubleRow pair layout: [t, p, ktp*256 + kt2*128 + j]
        #   = fp8(SCALE * s[t*128+j, (ktp*2+kt2)*128 + p])
        r = (SCALE * np.asarray(s, np.float32)).astype(NPFP8)
        r = r.reshape(NT, 128, NT, 128).transpose(0, 3, 2, 1)  # [t, p, kt, j]
        return np.ascontiguousarray(r.reshape(NT, 128, N))

    s0i, s1i = strip_img(support0), strip_img(support1)

    in_maps = []
    for c in range(NCORES):
        shard = xs[c * BS:(c + 1) * BS]                # [8b, N, F]
        # t-major image: x0t[f, t*BS*128 + b*128 + j] = shard[b, t*128+j, f];
        # fused as xw = [wcat | x0t] so one DMA feeds the first projection.
        x0t = shard.reshape(BS, NT, 128, F).transpose(3, 1, 0, 2).reshape(
            F, BS * N).astype(NPBF16)
        xw = np.ascontiguousarray(np.concatenate([wcat, x0t], axis=1))
        in_maps.append({"xw": xw, "s0t": s0i, "s1t": s1i})
    return in_maps


def postprocess(results, biases):
    full = np.empty((B, N, O), np.float32)
    for c, r in enumerate(results):
        full[c * BS:(c + 1) * BS] = (
            r["out"].astype(np.float32).reshape(N, BS, O).transpose(1, 0, 2)
        )
    full += np.asarray(biases, np.float32)[None, None, :]
    return full.reshape(B, N * O)


def kernel(support0, support1, inputs, state, weight, biases, output_size=None,
           **run_kwargs):
    nc = _get_nc()
    in_maps = make_inputs(support0, support1, inputs, state, weight)
    res = run_bass_kernel_spmd(nc, in_maps, core_ids=list(range(NCORES)),
                               **run_kwargs)
    out = postprocess(res.results, biases)
    if run_kwargs.get("trace"):
        return out, res
    return out
